# revision 21
# baseline (speedup 1.0000x reference)
"""Trainium2 Bass kernel for CombinedLoss (dice + hausdorff), 8-core SPMD.

Sharding: batch B=32 -> 4 samples/core, 12 (b,c) pairs per core.
Device computes, per (b,c):
    P = x @ y^T - 0.5*y2[j]        (PE, float32r matmuls + K=1 append matmul)
    row:  max_j P -> d_xy^2 = max_i (x2[i] - 2*max_j P[i,:])     (DVE)
    col:  Q = P - 0.5*x2[i] (ACT bias); d_yx^2 = -2*min_j max_i Q (GPSIMD)
Host does: input transposes (layout prep), x2/y2 row norms, dice term,
final sqrt/max/mean combine.
"""

import numpy as np

import concourse.bass as bass
import concourse.bacc as bacc
import concourse.mybir as mybir
import concourse.tile as tile
import concourse.bass_isa as bass_isa
from concourse.bass_utils import run_bass_kernel_spmd
from bass_rust import AxisListType

B, C, H, W = 32, 3, 512, 512
NCORES = 8
BPC = B // NCORES           # samples per core
NBC = BPC * C               # (b,c) pairs per core = 12
WEIGHT_DICE = 0.4
WEIGHT_HAUSDORFF = 0.6
SMOOTH = 1e-05

f32 = mybir.dt.float32
f32r = mybir.dt.float32r
ALU = mybir.AluOpType
ACTF = mybir.ActivationFunctionType

_CACHE = {}


def _build():
    nc = bacc.Bacc(None)
    # x and y stacked: xyt[bc, 0] = x^T, xyt[bc, 1] = y^T  (both w-major)
    xyt_d = nc.dram_tensor("xyt", [NBC, 2, W, H], f32r, kind="ExternalInput")
    # -0.5*x2 arranged [p, 4*bc+rb] so that i = 128*rb + p
    x2n_d = nc.dram_tensor("x2n", [128, NBC * 4], f32, kind="ExternalInput")
    # -0.5*y2 arranged [1, 512*bc + j]; last 128 entries are 1.0 (ones row)
    y2n_d = nc.dram_tensor("y2n", [1, NBC * H + 128], f32r, kind="ExternalInput")
    id_d = nc.dram_tensor("ident", [128, 128], f32, kind="ExternalInput")
    rrow_d = nc.dram_tensor("rrow", [128, NBC], f32, kind="ExternalOutput")
    rcol_d = nc.dram_tensor("rcol", [128, NBC], f32, kind="ExternalOutput")

    with tile.TileContext(nc) as tc:
        with (
            tc.tile_pool(name="const", bufs=1) as cpool,
            tc.tile_pool(name="xy", bufs=2) as xypool,
            tc.tile_pool(name="q", bufs=2) as qpool,
            tc.tile_pool(name="small", bufs=2) as spool,
            tc.tile_pool(name="psum", bufs=4, space="PSUM") as ppool,
            tc.tile_pool(name="psumt", bufs=2, space="PSUM") as tpool,
        ):
            x2n = cpool.tile([128, NBC * 4], f32, tag="x2n")
            nc.sync.dma_start(x2n[:], x2n_d[:])
            y2n = cpool.tile([1, NBC * H + 128], f32r, tag="y2n")
            nc.sync.dma_start(y2n[:], y2n_d[:])
            ones = y2n[0:1, NBC * H : NBC * H + 128]
            ident = cpool.tile([128, 128], f32, tag="ident")
            nc.sync.dma_start(ident[:], id_d[:])
            rrow = cpool.tile([128, NBC], f32, tag="rrow")
            rcol = cpool.tile([128, NBC], f32, tag="rcol")

            for bc in range(NBC):
                xyts = xypool.tile([128, 2 * 4 * H], f32r, tag="xyts")
                # xyts[p, 2048*t + 512*wb + i] = (x if t==0 else y)^T[128*wb + p, i]
                nc.sync.dma_start(
                    xyts[:].rearrange("p (t wb i) -> p t wb i", t=2, wb=4),
                    xyt_d[bc].rearrange("t (wb p) i -> p t wb i", p=128),
                )
                xts = xyts[:, 0 : 4 * H]
                yts = xyts[:, 4 * H : 8 * H]
                pm = spool.tile([128, 4], f32, tag="pm")
                qall = qpool.tile([128, 4 * H], f32, tag="qall")
                for rb in range(4):
                    P = ppool.tile([128, H], f32, tag="P")
                    for wb in range(4):
                        lo = 512 * wb + 128 * rb
                        nc.tensor.matmul(
                            P[:],
                            xts[:, lo : lo + 128],
                            yts[:, 512 * wb : 512 * wb + 512],
                            start=(wb == 0),
                            stop=False,
                        )
                    # P += ones^T @ (-0.5*y2 row)  -> P = G - 0.5*y2[j]
                    nc.tensor.matmul(
                        P[:],
                        ones,
                        y2n[0:1, H * bc : H * bc + H],
                        start=False,
                        stop=True,
                    )
                    # qall = P + (-0.5*x2[i]) = -0.5*d2   (sole PSUM reader)
                    nc.scalar.activation(
                        qall[:, H * rb : H * rb + H],
                        P[:],
                        ACTF.Identity,
                        bias=x2n[:, 4 * bc + rb : 4 * bc + rb + 1],
                        scale=1.0,
                    )
                # row path: d_xy^2 = -2 * min_i max_j qall
                for rb in range(4):
                    nc.vector.reduce_max(
                        pm[:, rb : rb + 1],
                        qall[:, H * rb : H * rb + H],
                        axis=AxisListType.X,
                    )
                nc.vector.tensor_reduce(
                    rrow[:, bc : bc + 1], pm[:], axis=AxisListType.X, op=ALU.min
                )
                # col path: max over rb blocks on gpsimd
                qm = qpool.tile([128, 2 * H], f32, tag="qm")
                nc.vector.tensor_tensor(
                    qm[:], qall[:, 0 : 2 * H], qall[:, 2 * H : 4 * H], op=ALU.max
                )
                qm2 = qpool.tile([128, H], f32, tag="qm2")
                nc.vector.tensor_tensor(
                    qm2[:], qm[:, 0:H], qm[:, H : 2 * H], op=ALU.max
                )
                # partition-axis max via PE transpose: T[jm, 128c+p] = qm2[p, 128c+jm]
                T = tpool.tile([128, H], f32, tag="T")
                for c4 in range(4):
                    nc.tensor.transpose(
                        T[:, 128 * c4 : 128 * c4 + 128],
                        qm2[:, 128 * c4 : 128 * c4 + 128],
                        ident[:],
                    )
                # max over p (innermost of free), then min over c; min over jm on host
                mt = spool.tile([128, 4], f32, tag="mt")
                nc.vector.tensor_reduce(
                    mt[:],
                    T[:].rearrange("jm (c p) -> jm c p", c=4),
                    axis=AxisListType.X,
                    op=ALU.max,
                )
                nc.vector.tensor_reduce(
                    rcol[:, bc : bc + 1], mt[:], axis=AxisListType.X, op=ALU.min
                )
            nc.sync.dma_start(rrow_d[:], rrow[:])
            nc.sync.dma_start(rcol_d[:], rcol[:])
    nc.finalize()
    return nc


def kernel(input, target, _stats=None):
    x = np.asarray(input, dtype=np.float32)
    y = np.asarray(target, dtype=np.float32)

    # ---- host: dice term ----
    xf = x.reshape(B, -1).astype(np.float64)
    yf = y.reshape(B, -1).astype(np.float64)
    inter = (xf * yf).sum(axis=1)
    union = xf.sum(axis=1) + yf.sum(axis=1)
    dice = float(np.mean(1.0 - (2.0 * inter + SMOOTH) / (union + SMOOTH)))

    # ---- host: layout prep for device ----
    xt = np.ascontiguousarray(x.transpose(0, 1, 3, 2))  # [B,C,W,H]
    yt = np.ascontiguousarray(y.transpose(0, 1, 3, 2))
    x2 = (x.astype(np.float64) ** 2).sum(axis=-1).astype(np.float32)  # [B,C,H]
    y2 = (y.astype(np.float64) ** 2).sum(axis=-1).astype(np.float32)

    in_maps = []
    for c in range(NCORES):
        b0 = c * BPC
        x2c = (-0.5 * x2[b0 : b0 + BPC]).reshape(NBC, 4, 128)
        x2n = np.ascontiguousarray(x2c.transpose(2, 0, 1)).reshape(128, NBC * 4)
        y2n = np.concatenate(
            [(-0.5 * y2[b0 : b0 + BPC]).reshape(NBC * H), np.ones(128, np.float32)]
        ).reshape(1, NBC * H + 128)
        xyt = np.stack(
            [
                xt[b0 : b0 + BPC].reshape(NBC, W, H),
                yt[b0 : b0 + BPC].reshape(NBC, W, H),
            ],
            axis=1,
        )
        in_maps.append(
            {
                "xyt": np.ascontiguousarray(xyt),
                "x2n": np.ascontiguousarray(x2n),
                "y2n": np.ascontiguousarray(y2n),
                "ident": np.eye(128, dtype=np.float32),
            }
        )

    if "nc" not in _CACHE:
        _CACHE["nc"] = _build()
    nc = _CACHE["nc"]

    import time as _time

    t0 = _time.time()
    br = run_bass_kernel_spmd(nc, in_maps, list(range(NCORES)), trace=False)
    t1 = _time.time()
    if isinstance(_stats, dict):
        _stats["wall_s"] = t1 - t0
        reps = _stats.get("repeats", 0)
        times = []
        for _ in range(reps):
            ta = _time.time()
            br = run_bass_kernel_spmd(nc, in_maps, list(range(NCORES)), trace=False)
            times.append(_time.time() - ta)
        _stats["repeat_wall_s"] = times

    # ---- host: combine ----
    hds = []
    for c in range(NCORES):
        rrow = br.results[c]["rrow"]  # [128, NBC]
        rcol = br.results[c]["rcol"]  # [128, NBC]
        dxy2 = -2.0 * rrow.min(axis=0)
        dyx2 = -2.0 * rcol.min(axis=0)
        hd2 = np.maximum(np.maximum(dxy2, dyx2), 0.0)
        hds.append(np.sqrt(hd2))
    hd = float(np.mean(np.concatenate(hds)))

    loss = WEIGHT_DICE * dice + WEIGHT_HAUSDORFF * hd
    return np.float32(loss)


# revision 31
# speedup vs baseline: 1.0937x; 1.0937x over previous
"""Trainium2 Bass kernel for CombinedLoss (dice + hausdorff), 8-core SPMD.

Sharding: batch B=32 -> 4 samples/core, 12 (b,c) pairs per core.
Device computes, per (b,c):
    P = x @ y^T - 0.5*y2[j]        (PE, float32r matmuls + K=1 append matmul)
    row:  max_j P -> d_xy^2 = max_i (x2[i] - 2*max_j P[i,:])     (DVE)
    col:  Q = P - 0.5*x2[i] (ACT bias); d_yx^2 = -2*min_j max_i Q (GPSIMD)
Host does: input transposes (layout prep), x2/y2 row norms, dice term,
final sqrt/max/mean combine.
"""

import numpy as np

import concourse.bass as bass
import concourse.bacc as bacc
import concourse.mybir as mybir
import concourse.tile as tile
import concourse.bass_isa as bass_isa
from concourse.bass_utils import run_bass_kernel_spmd
from bass_rust import AxisListType

B, C, H, W = 32, 3, 512, 512
NCORES = 8
BPC = B // NCORES           # samples per core
NBC = BPC * C               # (b,c) pairs per core = 12
WEIGHT_DICE = 0.4
WEIGHT_HAUSDORFF = 0.6
SMOOTH = 1e-05

f32 = mybir.dt.float32
f32r = mybir.dt.float32r
ALU = mybir.AluOpType
ACTF = mybir.ActivationFunctionType

_CACHE = {}


def _esel():
    e = np.zeros((NBC, NBC * 128), dtype=np.float32)
    for bc in range(NBC):
        e[bc, 128 * bc : 128 * bc + 128] = 1.0
    return e


def _build():
    nc = bacc.Bacc(None)
    # x and y stacked: xyt[bc, 0] = x^T, xyt[bc, 1] = y^T  (both w-major)
    xyt_d = nc.dram_tensor("xyt", [NBC, 2, W, H], f32r, kind="ExternalInput")
    # -0.5*x2 arranged [p, 4*bc+rb] so that i = 128*rb + p
    x2n_d = nc.dram_tensor("x2n", [128, NBC * 4], f32, kind="ExternalInput")
    # -0.5*y2: partition bc holds the y2 row for that bc
    y2n_d = nc.dram_tensor("y2n", [NBC, H], f32r, kind="ExternalInput")
    # one-hot selector: esel[k, 128*bc + i] = (k == bc), so
    # esel[:, 128*bc:+128].T @ y2n broadcasts y2 row bc over all partitions
    esel_d = nc.dram_tensor("esel", [NBC, NBC * 128], f32r, kind="ExternalInput")
    id_d = nc.dram_tensor("ident", [128, 128], f32, kind="ExternalInput")
    # res[:, 0:NBC] = rrow (row path), res[:, NBC:2*NBC] = rcol (col path)
    res_d = nc.dram_tensor("res", [128, 2 * NBC], f32, kind="ExternalOutput")

    with tile.TileContext(nc) as tc:
        with (
            tc.tile_pool(name="const", bufs=1) as cpool,
            tc.tile_pool(name="xy", bufs=3) as xypool,
            tc.tile_pool(name="q", bufs=2) as qpool,
            tc.tile_pool(name="small", bufs=2) as spool,
            tc.tile_pool(name="psum", bufs=1, space="PSUM") as ppool,
            tc.tile_pool(name="psumt", bufs=2, space="PSUM") as tpool,
        ):
            # prefetch bc0's first input chunk before the small constant
            # loads so the big stream starts immediately
            xyts0 = xypool.tile([128, 2 * 4 * H], f32r, tag="xyts", name="xyts_pre")
            nc.sync.dma_start(
                xyts0[:].rearrange("p (t wb i) -> p t wb i", t=2, wb=4)[:, :, 0, :],
                xyt_d[0, :, 0:128, :].rearrange("t p i -> p t i"),
            )
            x2n = cpool.tile([128, NBC * 4], f32, tag="x2n")
            nc.sync.dma_start(x2n[:], x2n_d[:])
            y2n = cpool.tile([NBC, H], f32r, tag="y2n")
            nc.sync.dma_start(y2n[:], y2n_d[:])
            esel = cpool.tile([NBC, NBC * 128], f32r, tag="esel")
            nc.sync.dma_start(esel[:], esel_d[:])
            ident = cpool.tile([128, 128], f32, tag="ident")
            nc.sync.dma_start(ident[:], id_d[:])
            res = cpool.tile([128, 2 * NBC], f32, tag="res")
            rrow = res[:, 0:NBC]
            rcol = res[:, NBC : 2 * NBC]

            for bc in range(NBC):
                if bc == 0:
                    xyts = xyts0
                else:
                    xyts = xypool.tile([128, 2 * 4 * H], f32r, tag="xyts")
                # xyts[p, 2048*t + 512*wb + i] = (x if t==0 else y)^T[128*wb + p, i]
                # one DMA per wb chunk so wb=0 matmuls start after ~1MB
                for wb in range(1 if bc == 0 else 0, 4):
                    nc.sync.dma_start(
                        xyts[:].rearrange("p (t wb i) -> p t wb i", t=2, wb=4)[
                            :, :, wb, :
                        ],
                        xyt_d[bc, :, 128 * wb : 128 * wb + 128, :].rearrange(
                            "t p i -> p t i"
                        ),
                    )
                xts = xyts[:, 0 : 4 * H]
                yts = xyts[:, 4 * H : 8 * H]
                pm = spool.tile([128, 4], f32, tag="pm")
                qall = qpool.tile([128, 4 * H], f32, tag="qall")
                # P_rb = G_rb - 0.5*y2[j]; wb-outer emission so only the
                # final-wb matmuls trail the last input chunk; the y2 append
                # rides just after wb0 (it only needs y2n, on-chip from t=0).
                Ps = [ppool.tile([128, H], f32, tag=f"P{i}", name=f"P{bc}_{i}") for i in range(4)]
                for wb in range(4):
                    for rb in range(4):
                        lo = 512 * wb + 128 * rb
                        nc.tensor.matmul(
                            Ps[rb][:],
                            xts[:, lo : lo + 128],
                            yts[:, 512 * wb : 512 * wb + 512],
                            start=(wb == 0),
                            stop=(wb == 3),
                        )
                        if wb == 0:
                            nc.tensor.matmul(
                                Ps[rb][:],
                                esel[:, 128 * bc : 128 * bc + 128],
                                y2n[:],
                                start=False,
                                stop=False,
                            )
                qm2 = qpool.tile([128, H], f32, tag="qm2")
                for rb in range(4):
                    # qall = P + (-0.5*x2[i]) = -0.5*d2   (sole PSUM reader)
                    nc.scalar.activation(
                        qall[:, H * rb : H * rb + H],
                        Ps[rb][:],
                        ACTF.Identity,
                        bias=x2n[:, 4 * bc + rb : 4 * bc + rb + 1],
                        scale=1.0,
                    )
                    # row path: pm[:, rb] = max_j Q_rb
                    nc.vector.reduce_max(
                        pm[:, rb : rb + 1],
                        qall[:, H * rb : H * rb + H],
                        axis=AxisListType.X,
                    )
                    # col path: incremental max over rb blocks
                    if rb == 1:
                        nc.vector.tensor_tensor(
                            qm2[:], qall[:, 0:H], qall[:, H : 2 * H], op=ALU.max
                        )
                    elif rb > 1:
                        nc.vector.tensor_tensor(
                            qm2[:],
                            qm2[:],
                            qall[:, H * rb : H * rb + H],
                            op=ALU.max,
                        )
                # row: d_xy^2 = -2 * min_i max_j qall
                nc.vector.tensor_reduce(
                    rrow[:, bc : bc + 1], pm[:], axis=AxisListType.X, op=ALU.min
                )
                # partition-axis max via PE transpose: T[jm, 128c+p] = qm2[p, 128c+jm]
                T = tpool.tile([128, H], f32, tag="T", name=f"T{bc}")
                for c4 in range(4):
                    nc.tensor.transpose(
                        T[:, 128 * c4 : 128 * c4 + 128],
                        qm2[:, 128 * c4 : 128 * c4 + 128],
                        ident[:],
                    )
                # max over p (innermost of free), then min over c; min over jm on host
                mt = spool.tile([128, 4], f32, tag="mt")
                nc.vector.tensor_reduce(
                    mt[:],
                    T[:].rearrange("jm (c p) -> jm c p", c=4),
                    axis=AxisListType.X,
                    op=ALU.max,
                )
                nc.vector.tensor_reduce(
                    rcol[:, bc : bc + 1], mt[:], axis=AxisListType.X, op=ALU.min
                )
            nc.sync.dma_start(res_d[:], res[:])
    nc.finalize()
    return nc


def kernel(input, target, _stats=None):
    x = np.asarray(input, dtype=np.float32)
    y = np.asarray(target, dtype=np.float32)

    # ---- host: dice term ----
    xf = x.reshape(B, -1).astype(np.float64)
    yf = y.reshape(B, -1).astype(np.float64)
    inter = (xf * yf).sum(axis=1)
    union = xf.sum(axis=1) + yf.sum(axis=1)
    dice = float(np.mean(1.0 - (2.0 * inter + SMOOTH) / (union + SMOOTH)))

    # ---- host: layout prep for device ----
    xt = np.ascontiguousarray(x.transpose(0, 1, 3, 2))  # [B,C,W,H]
    yt = np.ascontiguousarray(y.transpose(0, 1, 3, 2))
    x2 = (x.astype(np.float64) ** 2).sum(axis=-1).astype(np.float32)  # [B,C,H]
    y2 = (y.astype(np.float64) ** 2).sum(axis=-1).astype(np.float32)

    in_maps = []
    for c in range(NCORES):
        b0 = c * BPC
        x2c = (-0.5 * x2[b0 : b0 + BPC]).reshape(NBC, 4, 128)
        x2n = np.ascontiguousarray(x2c.transpose(2, 0, 1)).reshape(128, NBC * 4)
        y2n = (-0.5 * y2[b0 : b0 + BPC]).reshape(NBC, H)
        xyt = np.stack(
            [
                xt[b0 : b0 + BPC].reshape(NBC, W, H),
                yt[b0 : b0 + BPC].reshape(NBC, W, H),
            ],
            axis=1,
        )
        in_maps.append(
            {
                "xyt": np.ascontiguousarray(xyt),
                "x2n": np.ascontiguousarray(x2n),
                "y2n": np.ascontiguousarray(y2n),
                "ident": np.eye(128, dtype=np.float32),
                "esel": _esel(),
            }
        )

    if "nc" not in _CACHE:
        _CACHE["nc"] = _build()
    nc = _CACHE["nc"]

    import time as _time

    t0 = _time.time()
    br = run_bass_kernel_spmd(nc, in_maps, list(range(NCORES)), trace=False)
    t1 = _time.time()
    if isinstance(_stats, dict):
        _stats["wall_s"] = t1 - t0
        reps = _stats.get("repeats", 0)
        times = []
        for _ in range(reps):
            ta = _time.time()
            br = run_bass_kernel_spmd(nc, in_maps, list(range(NCORES)), trace=False)
            times.append(_time.time() - ta)
        _stats["repeat_wall_s"] = times

    # ---- host: combine ----
    hds = []
    for c in range(NCORES):
        res = br.results[c]["res"]  # [128, 2*NBC]
        rrow = res[:, :NBC]
        rcol = res[:, NBC:]
        dxy2 = -2.0 * rrow.min(axis=0)
        dyx2 = -2.0 * rcol.min(axis=0)
        hd2 = np.maximum(np.maximum(dxy2, dyx2), 0.0)
        hds.append(np.sqrt(hd2))
    hd = float(np.mean(np.concatenate(hds)))

    loss = WEIGHT_DICE * dice + WEIGHT_HAUSDORFF * hd
    return np.float32(loss)


# revision 32
# speedup vs baseline: 37160.0223x; 33976.2438x over previous
"""Trainium2 Bass kernel for CombinedLoss (dice + hausdorff), 8-core SPMD.

Sharding: batch B=32 -> 4 samples/core, 12 (b,c) pairs per core.
Device computes, per (b,c):
    P = x @ y^T - 0.5*y2[j]        (PE, float32r matmuls + K=1 append matmul)
    row:  max_j P -> d_xy^2 = max_i (x2[i] - 2*max_j P[i,:])     (DVE)
    col:  Q = P - 0.5*x2[i] (ACT bias); d_yx^2 = -2*min_j max_i Q (GPSIMD)
Host does: input transposes (layout prep), x2/y2 row norms, dice term,
final sqrt/max/mean combine.
"""

import numpy as np

import concourse.bass as bass
import concourse.bacc as bacc
import concourse.mybir as mybir
import concourse.tile as tile
import concourse.bass_isa as bass_isa
from concourse.bass_utils import run_bass_kernel_spmd
from bass_rust import AxisListType

B, C, H, W = 32, 3, 512, 512
NCORES = 8
BPC = B // NCORES           # samples per core
NBC = BPC * C               # (b,c) pairs per core = 12
WEIGHT_DICE = 0.4
WEIGHT_HAUSDORFF = 0.6
SMOOTH = 1e-05

f32 = mybir.dt.float32
f32r = mybir.dt.float32r
ALU = mybir.AluOpType
ACTF = mybir.ActivationFunctionType

_CACHE = {}
_UID = [0]


def _uid():
    _UID[0] += 1
    return _UID[0]


def _esel():
    e = np.zeros((NBC, NBC * 128), dtype=np.float32)
    for bc in range(NBC):
        e[bc, 128 * bc : 128 * bc + 128] = 1.0
    return e


def _build(repeat=1):
    nc = bacc.Bacc(None)
    # x and y stacked: xyt[bc, 0] = x^T, xyt[bc, 1] = y^T  (both w-major)
    xyt_d = nc.dram_tensor("xyt", [NBC, 2, W, H], f32r, kind="ExternalInput")
    # -0.5*x2 arranged [p, 4*bc+rb] so that i = 128*rb + p
    x2n_d = nc.dram_tensor("x2n", [128, NBC * 4], f32, kind="ExternalInput")
    # -0.5*y2: partition bc holds the y2 row for that bc
    y2n_d = nc.dram_tensor("y2n", [NBC, H], f32r, kind="ExternalInput")
    # one-hot selector: esel[k, 128*bc + i] = (k == bc), so
    # esel[:, 128*bc:+128].T @ y2n broadcasts y2 row bc over all partitions
    esel_d = nc.dram_tensor("esel", [NBC, NBC * 128], f32r, kind="ExternalInput")
    id_d = nc.dram_tensor("ident", [128, 128], f32, kind="ExternalInput")
    # res[:, 0:NBC] = rrow (row path), res[:, NBC:2*NBC] = rcol (col path)
    res_d = nc.dram_tensor("res", [128, 2 * NBC], f32, kind="ExternalOutput")

    with tile.TileContext(nc) as tc:
        with (
            tc.tile_pool(name="const", bufs=1) as cpool,
            tc.tile_pool(name="xy", bufs=3) as xypool,
            tc.tile_pool(name="q", bufs=2) as qpool,
            tc.tile_pool(name="small", bufs=2) as spool,
            tc.tile_pool(name="psum", bufs=1, space="PSUM") as ppool,
            tc.tile_pool(name="psumt", bufs=2, space="PSUM") as tpool,
        ):
            # prefetch bc0's first input chunk before the small constant
            # loads so the big stream starts immediately
            xyts0 = xypool.tile([128, 2 * 4 * H], f32r, tag="xyts", name="xyts_pre")
            nc.sync.dma_start(
                xyts0[:].rearrange("p (t wb i) -> p t wb i", t=2, wb=4)[:, :, 0, :],
                xyt_d[0, :, 0:128, :].rearrange("t p i -> p t i"),
            )
            x2n = cpool.tile([128, NBC * 4], f32, tag="x2n")
            nc.sync.dma_start(x2n[:], x2n_d[:])
            y2n = cpool.tile([NBC, H], f32r, tag="y2n")
            nc.sync.dma_start(y2n[:], y2n_d[:])
            esel = cpool.tile([NBC, NBC * 128], f32r, tag="esel")
            nc.sync.dma_start(esel[:], esel_d[:])
            ident = cpool.tile([128, 128], f32, tag="ident")
            nc.sync.dma_start(ident[:], id_d[:])
            res = cpool.tile([128, 2 * NBC], f32, tag="res")
            rrow = res[:, 0:NBC]
            rcol = res[:, NBC : 2 * NBC]

            for bc in [b for _ in range(repeat) for b in range(NBC)]:
                if bc == 0:
                    xyts = xyts0
                else:
                    xyts = xypool.tile([128, 2 * 4 * H], f32r, tag="xyts")
                # xyts[p, 2048*t + 512*wb + i] = (x if t==0 else y)^T[128*wb + p, i]
                # one DMA per wb chunk so wb=0 matmuls start after ~1MB
                for wb in range(1 if bc == 0 else 0, 4):
                    nc.sync.dma_start(
                        xyts[:].rearrange("p (t wb i) -> p t wb i", t=2, wb=4)[
                            :, :, wb, :
                        ],
                        xyt_d[bc, :, 128 * wb : 128 * wb + 128, :].rearrange(
                            "t p i -> p t i"
                        ),
                    )
                xts = xyts[:, 0 : 4 * H]
                yts = xyts[:, 4 * H : 8 * H]
                pm = spool.tile([128, 4], f32, tag="pm")
                qall = qpool.tile([128, 4 * H], f32, tag="qall")
                # P_rb = G_rb - 0.5*y2[j]; wb-outer emission so only the
                # final-wb matmuls trail the last input chunk; the y2 append
                # rides just after wb0 (it only needs y2n, on-chip from t=0).
                Ps = [ppool.tile([128, H], f32, tag=f"P{i}", name=f"P{_uid()}_{i}") for i in range(4)]
                for wb in range(4):
                    for rb in range(4):
                        lo = 512 * wb + 128 * rb
                        nc.tensor.matmul(
                            Ps[rb][:],
                            xts[:, lo : lo + 128],
                            yts[:, 512 * wb : 512 * wb + 512],
                            start=(wb == 0),
                            stop=(wb == 3),
                        )
                        if wb == 0:
                            nc.tensor.matmul(
                                Ps[rb][:],
                                esel[:, 128 * bc : 128 * bc + 128],
                                y2n[:],
                                start=False,
                                stop=False,
                            )
                qm2 = qpool.tile([128, H], f32, tag="qm2")
                for rb in range(4):
                    # qall = P + (-0.5*x2[i]) = -0.5*d2   (sole PSUM reader)
                    nc.scalar.activation(
                        qall[:, H * rb : H * rb + H],
                        Ps[rb][:],
                        ACTF.Identity,
                        bias=x2n[:, 4 * bc + rb : 4 * bc + rb + 1],
                        scale=1.0,
                    )
                    # row path: pm[:, rb] = max_j Q_rb
                    nc.vector.reduce_max(
                        pm[:, rb : rb + 1],
                        qall[:, H * rb : H * rb + H],
                        axis=AxisListType.X,
                    )
                    # col path: incremental max over rb blocks
                    if rb == 1:
                        nc.vector.tensor_tensor(
                            qm2[:], qall[:, 0:H], qall[:, H : 2 * H], op=ALU.max
                        )
                    elif rb > 1:
                        nc.vector.tensor_tensor(
                            qm2[:],
                            qm2[:],
                            qall[:, H * rb : H * rb + H],
                            op=ALU.max,
                        )
                # row: d_xy^2 = -2 * min_i max_j qall
                nc.vector.tensor_reduce(
                    rrow[:, bc : bc + 1], pm[:], axis=AxisListType.X, op=ALU.min
                )
                # partition-axis max via PE transpose: T[jm, 128c+p] = qm2[p, 128c+jm]
                T = tpool.tile([128, H], f32, tag="T", name=f"T{_uid()}")
                for c4 in range(4):
                    nc.tensor.transpose(
                        T[:, 128 * c4 : 128 * c4 + 128],
                        qm2[:, 128 * c4 : 128 * c4 + 128],
                        ident[:],
                    )
                # max over p (innermost of free), then min over c; min over jm on host
                mt = spool.tile([128, 4], f32, tag="mt")
                nc.vector.tensor_reduce(
                    mt[:],
                    T[:].rearrange("jm (c p) -> jm c p", c=4),
                    axis=AxisListType.X,
                    op=ALU.max,
                )
                nc.vector.tensor_reduce(
                    rcol[:, bc : bc + 1], mt[:], axis=AxisListType.X, op=ALU.min
                )
            nc.sync.dma_start(res_d[:], res[:])
    nc.finalize()
    return nc


def kernel(input, target, _stats=None):
    x = np.asarray(input, dtype=np.float32)
    y = np.asarray(target, dtype=np.float32)

    # ---- host: dice term ----
    xf = x.reshape(B, -1).astype(np.float64)
    yf = y.reshape(B, -1).astype(np.float64)
    inter = (xf * yf).sum(axis=1)
    union = xf.sum(axis=1) + yf.sum(axis=1)
    dice = float(np.mean(1.0 - (2.0 * inter + SMOOTH) / (union + SMOOTH)))

    # ---- host: layout prep for device ----
    xt = np.ascontiguousarray(x.transpose(0, 1, 3, 2))  # [B,C,W,H]
    yt = np.ascontiguousarray(y.transpose(0, 1, 3, 2))
    x2 = (x.astype(np.float64) ** 2).sum(axis=-1).astype(np.float32)  # [B,C,H]
    y2 = (y.astype(np.float64) ** 2).sum(axis=-1).astype(np.float32)

    in_maps = []
    for c in range(NCORES):
        b0 = c * BPC
        x2c = (-0.5 * x2[b0 : b0 + BPC]).reshape(NBC, 4, 128)
        x2n = np.ascontiguousarray(x2c.transpose(2, 0, 1)).reshape(128, NBC * 4)
        y2n = (-0.5 * y2[b0 : b0 + BPC]).reshape(NBC, H)
        xyt = np.stack(
            [
                xt[b0 : b0 + BPC].reshape(NBC, W, H),
                yt[b0 : b0 + BPC].reshape(NBC, W, H),
            ],
            axis=1,
        )
        in_maps.append(
            {
                "xyt": np.ascontiguousarray(xyt),
                "x2n": np.ascontiguousarray(x2n),
                "y2n": np.ascontiguousarray(y2n),
                "ident": np.eye(128, dtype=np.float32),
                "esel": _esel(),
            }
        )

    if "nc" not in _CACHE:
        _CACHE["nc"] = _build()
    nc = _CACHE["nc"]

    import time as _time

    t0 = _time.time()
    br = run_bass_kernel_spmd(nc, in_maps, list(range(NCORES)), trace=False)
    t1 = _time.time()
    if isinstance(_stats, dict):
        _stats["wall_s"] = t1 - t0
        reps = _stats.get("repeats", 0)
        times = []
        for _ in range(reps):
            ta = _time.time()
            br = run_bass_kernel_spmd(nc, in_maps, list(range(NCORES)), trace=False)
            times.append(_time.time() - ta)
        _stats["repeat_wall_s"] = times

    # ---- host: combine ----
    hds = []
    for c in range(NCORES):
        res = br.results[c]["res"]  # [128, 2*NBC]
        rrow = res[:, :NBC]
        rcol = res[:, NBC:]
        dxy2 = -2.0 * rrow.min(axis=0)
        dyx2 = -2.0 * rcol.min(axis=0)
        hd2 = np.maximum(np.maximum(dxy2, dyx2), 0.0)
        hds.append(np.sqrt(hd2))
    hd = float(np.mean(np.concatenate(hds)))

    loss = WEIGHT_DICE * dice + WEIGHT_HAUSDORFF * hd
    return np.float32(loss)
